# revision 1
# baseline (speedup 1.0000x reference)
"""Trainium2 Bass kernel v3 for DepthSeparableConv2d (dw3x3+BN+ReLU+cut, pw1x1+BN+ReLU+cut).

Contract: kernel(**inputs) takes FULL unsharded numpy inputs, returns FULL
[32, 128, 112, 112] float32 output. Data-parallel over batch: 4 samples/core
on 8 NeuronCores; per-core 2 blocks of (2 samples x 64 ch) = 128 partitions.

Design (engine-balanced from HW-measured rates; DVE ops are 1x = 0.96 GHz
except tensor_scalar which reaches 2x via the two-port mode):
 - dw 3x3: 23 groups (8 rows) on PE as diagonal-weight accumulating matmuls
   (3.46us/group) drained by ACT (relu+bias, 1.0us); 5 groups on DVE as
   scalar_tensor_tensor chains, processed as 16-row doubles where possible.
 - channel-cut-1 plane maxes: DVE tensor_reduce per group/double.
 - pw 1x1: K=64 matmul per sample (serialized pairs, output-port bound);
   drains are [2 samples, 448] pair-ops split ACT/DVE (ACT-heavy while DVE
   is busy with dw, even split in the tail); z streamed to HBM as f16.
 - channel-cut-2 omitted (cannot trigger: needs an entire post-relu plane
   < 1e-3).
 - bn1/bn2 folded into weights/biases on host.
"""

import numpy as np
import ml_dtypes

B, C_IN, C_OUT, H, W = 32, 64, 128, 112, 112
HP, WP = H + 2, W + 2
EPS = 1e-5
DW_THRESH = 4.0
N_CORES = 8
SPC = B // N_CORES          # samples per core = 4
BLOCKS = SPC // 2           # 2
HW = H * W                  # 12544
CHUNK = 4 * W               # 448 (one PSUM bank)
GROUPS = H // 8             # 14 groups of 8 rows per plane

# DVE dw assignment per block: list of (start_group, n_groups) runs
DVE_RUNS = [[(12, 2)], [(11, 2), (13, 1)]]

_CACHE = {}


def _build_bass():
    import concourse.bass as bass
    import concourse.tile as tile
    from concourse import bacc, mybir
    from contextlib import ExitStack

    f32 = mybir.dt.float32
    f16 = mybir.dt.float16
    Alu = mybir.AluOpType
    Act = mybir.ActivationFunctionType

    nc = bacc.Bacc("TRN2", target_bir_lowering=False, debug=False)

    X = nc.dram_tensor("xp", [BLOCKS, 128, HP, WP], f16, kind="ExternalInput")
    WDW = nc.dram_tensor("wdw", [128, 9, 128], f16, kind="ExternalInput")
    WPW = nc.dram_tensor("wpw", [128, 128], f16, kind="ExternalInput")
    WV = nc.dram_tensor("wv", [128, 9], f32, kind="ExternalInput")
    B1 = nc.dram_tensor("b1", [128, 1], f32, kind="ExternalInput")
    B2 = nc.dram_tensor("b2", [128, 1], f32, kind="ExternalInput")
    Z = nc.dram_tensor("z", [SPC, 128, HW], f16, kind="ExternalOutput")

    Xap = X.ap()
    Zap = Z.ap()

    with ExitStack() as ctx:
        tc = ctx.enter_context(tile.TileContext(nc))
        consts = ctx.enter_context(tc.tile_pool(name="consts", bufs=1))
        xpool = ctx.enter_context(tc.tile_pool(name="xpool", bufs=2))
        ypool = ctx.enter_context(tc.tile_pool(name="ypool", bufs=2))
        zpool = ctx.enter_context(tc.tile_pool(name="zpool", bufs=3))
        small = ctx.enter_context(tc.tile_pool(name="small", bufs=4))
        accpool = ctx.enter_context(tc.tile_pool(name="accpool", bufs=2))
        wmpool = ctx.enter_context(tc.tile_pool(name="wmpool", bufs=2))
        psdw = ctx.enter_context(tc.tile_pool(name="psdw", bufs=2, space="PSUM"))
        pspw = ctx.enter_context(tc.tile_pool(name="pspw", bufs=2, space="PSUM"))

        wdw_t = consts.tile([128, 9, 128], f16)
        nc.sync.dma_start(out=wdw_t, in_=WDW.ap())
        wpw_t = consts.tile([128, 128], f16)
        nc.sync.dma_start(out=wpw_t, in_=WPW.ap())
        wv_t = consts.tile([128, 9], f32)
        nc.sync.dma_start(out=wv_t, in_=WV.ap())
        b1_t = consts.tile([128, 1], f32)
        nc.sync.dma_start(out=b1_t, in_=B1.ap())
        b2_t = consts.tile([128, 1], f32)
        nc.sync.dma_start(out=b2_t, in_=B2.ap())

        xts = [None] * BLOCKS
        yts = [None] * BLOCKS
        m1cs = [None] * BLOCKS
        wms = [None] * BLOCKS
        pw_dve = {"phase": 0}  # 0: dw-heavy phase (ACT-biased), 1: tail (even)
        pw_cnt = {"n": 0}

        def load_x(blk):
            xt = xpool.tile([128, HP, WP], f16, tag="x", name=f"xt{blk}")
            for r0, r1 in ((0, 10), (10, 30), (30, 58), (58, 86), (86, HP)):
                nc.sync.dma_start(out=xt[:, r0:r1, :], in_=Xap[blk, :, r0:r1, :])
            xts[blk] = xt
            yts[blk] = ypool.tile([128, H, W], f16, tag="y", name=f"yt{blk}")
            m1cs[blk] = small.tile([128, GROUPS], f32, tag="m1c", name=f"m1c{blk}")

        def dw_group_pe(blk, g):
            # depthwise 3x3 for output rows [8g, 8g+8) via diagonal matmuls
            xt, yt = xts[blk], yts[blk]
            ps = psdw.tile([128, 2, 512], f32, tag="psdw", name=f"psdw{blk}_{g}")
            for tap in range(9):
                dr, dc = divmod(tap, 3)
                for j in range(2):
                    r0 = g * 8 + j * 4
                    nc.tensor.matmul(
                        ps[:, j, 0:CHUNK],
                        lhsT=wdw_t[:, tap, :],
                        rhs=xt[:, r0 + dr: r0 + dr + 4, dc: dc + W],
                        start=(tap == 0),
                        stop=(tap == 8),
                        skip_group_check=True,
                    )
            # ACT drain: y = relu(psum + b1), downcast fp16
            nc.scalar.activation(
                yt[:, g * 8: (g + 1) * 8, :],
                ps[:, :, 0:CHUNK],
                Act.Relu,
                bias=b1_t[:, :],
                scale=1.0,
            )

        def max_red(blk, g):
            # chunk max of post-relu y for channel-cut-1
            def op():
                nc.vector.tensor_reduce(
                    m1cs[blk][:, g: g + 1],
                    yts[blk][:, g * 8: (g + 1) * 8, :],
                    axis=mybir.AxisListType.XY,
                    op=Alu.max)
            return op

        def dw_run_dve_ops(blk, g0, ng):
            """Closures for `ng` consecutive dw groups ([8*g0, 8*(g0+ng)) rows)
            as one DVE op-chain: 9-tap STT chain (1x), tensor_scalar relu (2x),
            one fused max reduce into m1c[:, g0]."""
            xt, yt = xts[blk], yts[blk]
            r0 = g0 * 8
            rows = ng * 8
            acc = accpool.tile([128, rows, W], f16, tag=f"acc{ng}",
                               name=f"acc{blk}_{g0}")
            ops = []

            def first_tap():  # acc = x*w + b1   (tap (0,0))
                nc.vector.tensor_scalar(
                    out=acc, in0=xt[:, r0: r0 + rows, 0: W],
                    scalar1=wv_t[:, 0:1], scalar2=b1_t[:, :],
                    op0=Alu.mult, op1=Alu.add)
            ops.append(first_tap)
            for tap in range(1, 9):
                dr, dc = divmod(tap, 3)

                def mac(dr=dr, dc=dc, tap=tap):
                    nc.vector.scalar_tensor_tensor(
                        out=acc, in0=xt[:, r0 + dr: r0 + dr + rows, dc: dc + W],
                        scalar=wv_t[:, tap: tap + 1],
                        in1=acc, op0=Alu.mult, op1=Alu.add)
                ops.append(mac)

            def relu():
                nc.vector.tensor_scalar(
                    out=yt[:, r0: r0 + rows, :], in0=acc,
                    scalar1=0.0, scalar2=None, op0=Alu.max)
            ops.append(relu)

            def red():
                nc.vector.tensor_reduce(
                    m1cs[blk][:, g0: g0 + 1],
                    yt[:, r0: r0 + rows, :],
                    axis=mybir.AxisListType.XY,
                    op=Alu.max)
                # unused m1c slots of the run: fill with the same reduce target
                for g in range(g0 + 1, g0 + ng):
                    nc.vector.tensor_scalar(
                        out=m1cs[blk][:, g: g + 1],
                        in0=m1cs[blk][:, g0: g0 + 1],
                        scalar1=1.0, scalar2=None, op0=Alu.mult)
            ops.append(red)
            return ops

        def finish_mask(blk):
            m1 = small.tile([128, 1], f32, tag="m1", name=f"m1_{blk}")
            nc.vector.tensor_reduce(
                m1, m1cs[blk], axis=mybir.AxisListType.X, op=Alu.max)
            mask1 = small.tile([128, 1], f32, tag="mask1", name=f"mask1_{blk}")
            nc.vector.tensor_scalar(
                out=mask1, in0=m1, scalar1=DW_THRESH, scalar2=None,
                op0=Alu.is_ge)
            wm = wmpool.tile([128, 128], f16, tag="wm", name=f"wm{blk}")
            nc.vector.tensor_scalar_mul(wm, wpw_t, mask1)
            wms[blk] = wm

        def pw_stage(blk, st):
            """pw conv + relu for groups [st, st+2): 4 chunk-pairs; each pair
            = 2 matmuls (s0 rows 0-63, s1 rows 64-127) into one psum tile,
            drained in ONE [2,448] op (ACT or DVE); 2 DMA stores per stage."""
            yflat = yts[blk].rearrange("p a b -> p (a b)")
            ngr = min(2, GROUPS - st)
            zs = zpool.tile([128, 2, 2 * ngr, CHUNK], f16, tag="zst",
                            name=f"zst{blk}_{st}")
            for i in range(2 * ngr):
                g, j = st + i // 2, i % 2
                off = (2 * g + j) * CHUNK
                pool = psdw if (pw_dve["phase"] == 1 and i % 2) else pspw
                pp = pool.tile([128, 2, 512], f32,
                               tag="psdw" if pool is psdw else "pspw",
                               name=f"pspw{blk}_{g}_{j}")
                for s in range(2):
                    nc.tensor.matmul(
                        pp[:, s, 0:CHUNK],
                        lhsT=wms[blk][64 * s: 64 * s + 64, :],
                        rhs=yflat[64 * s: 64 * s + 64, off: off + CHUNK],
                        start=True,
                        stop=True,
                    )
                pw_cnt["n"] += 1
                to_dve = False if pw_dve["phase"] == 0 \
                    else (pw_cnt["n"] % 15 in (0, 2, 4, 6, 8, 10, 12))
                if to_dve:
                    nc.vector.tensor_scalar(
                        out=zs[:, :, i, :], in0=pp[:, :, 0:CHUNK],
                        scalar1=b2_t[:, :], scalar2=0.0,
                        op0=Alu.add, op1=Alu.max)
                else:
                    nc.scalar.activation(
                        zs[:, :, i, :], pp[:, :, 0:CHUNK],
                        Act.Relu, bias=b2_t[:, :], scale=1.0)
            for s in range(2):
                smp = blk * 2 + s
                nc.sync.dma_start(
                    out=Zap[smp, :, 2 * st * CHUNK: 2 * (st + ngr) * CHUNK],
                    in_=zs[:, s, :, :].rearrange("p a b -> p (a b)"),
                )

        def emit_dw_block(blk, extra_every=None, extra=None):
            """Emit block blk: PE groups inline; DVE run op-chains and PE-group
            maxes paced between them; optional pw stages of the previous block
            after given PE-group indices."""
            chain = []
            dve_groups = set()
            for g0, ng in DVE_RUNS[blk]:
                chain.extend(dw_run_dve_ops(blk, g0, ng))
                dve_groups.update(range(g0, g0 + ng))
            pe_groups = [g for g in range(GROUPS) if g not in dve_groups]
            post = {}
            for i, g in enumerate(pe_groups):
                post.setdefault(i + 1, []).append(max_red(blk, g))
            per = (len(chain) + len(pe_groups) - 1) // len(pe_groups)
            ci = 0
            for i, g in enumerate(pe_groups):
                dw_group_pe(blk, g)
                for _ in range(per):
                    if ci < len(chain):
                        chain[ci]()
                        ci += 1
                for op in post.get(i, []):
                    op()
                if extra_every and i in extra_every:
                    extra(extra_every[i])
            while ci < len(chain):
                chain[ci]()
                ci += 1
            for op in post.get(len(pe_groups), []):
                op()

        # ---- PE warmup: junk matmuls from t~0 so HAM reaches 8/8 ----
        warm_src = consts.tile([128, 512], f16)
        nc.vector.memset(warm_src, 0.5)
        for w in range(10):
            wps = pspw.tile([128, 2, 512], f32, tag="pspw", name=f"warm{w}")
            nc.tensor.matmul(
                wps[:, 0, 0:512], lhsT=warm_src[:, 0:128], rhs=warm_src,
                start=True, stop=True)

        # ---- emission ----
        load_x(0)
        load_x(1)
        emit_dw_block(0)
        finish_mask(0)
        emit_dw_block(1, extra_every={1: 0, 3: 2, 5: 4, 6: 6, 8: 8, 9: 10,
                                      10: 12},
                      extra=lambda st: pw_stage(0, st))
        finish_mask(1)
        pw_dve["phase"] = 1
        for st in range(0, GROUPS, 2):
            pw_stage(1, st)

    nc.finalize()
    return nc


def _get_nc():
    if "nc" not in _CACHE:
        _CACHE["nc"] = _build_bass()
    return _CACHE["nc"]


def _prepare_inputs(x, dw_w, dw_b, bn1_g, bn1_b, bn1_m, bn1_v,
                    pw_w, pw_b, bn2_g, bn2_b, bn2_m, bn2_v):
    f8 = np.float64
    inv1 = bn1_g.astype(f8) / np.sqrt(bn1_v.astype(f8) + EPS)
    w1 = dw_w.astype(f8)[:, 0] * inv1[:, None, None]          # [64,3,3]
    b1 = (dw_b.astype(f8) - bn1_m.astype(f8)) * inv1 + bn1_b.astype(f8)
    inv2 = bn2_g.astype(f8) / np.sqrt(bn2_v.astype(f8) + EPS)
    w2 = pw_w.astype(f8) * inv2[:, None]                      # [128(o),64(c)]
    b2 = (pw_b.astype(f8) - bn2_m.astype(f8)) * inv2 + bn2_b.astype(f8)

    w1f = w1.reshape(64, 9).astype(np.float32)                # [c, tap]
    wdw = np.zeros((128, 9, 128), dtype=np.float32)
    idx = np.arange(128)
    wdw[idx, :, idx] = w1f[idx % 64, :]
    wdw = wdw.astype(np.float16)
    wv = wdw[np.arange(128), :, np.arange(128)].astype(np.float32)  # [128, 9]

    wpw = np.ascontiguousarray(
        w2.astype(np.float32).T[np.arange(128) % 64, :]
    ).astype(np.float16)                                      # [128, 128]

    b1_dup = b1.astype(np.float32)[np.arange(128) % 64].reshape(128, 1)
    b2_arr = b2.astype(np.float32).reshape(128, 1)

    xpad = np.zeros((B, C_IN, HP, WP), dtype=np.float16)
    xpad[:, :, 1:1 + H, 1:1 + W] = x.astype(np.float16)

    in_maps = []
    for c in range(N_CORES):
        xc = xpad[SPC * c: SPC * (c + 1)].reshape(BLOCKS, 128, HP, WP)
        in_maps.append({
            "xp": np.ascontiguousarray(xc),
            "wdw": wdw,
            "wv": wv,
            "wpw": wpw,
            "b1": b1_dup,
            "b2": b2_arr,
        })
    return in_maps


def _run(in_maps, **kw):
    from concourse import bass_utils
    nc = _get_nc()
    return bass_utils.run_bass_kernel_spmd(
        nc, in_maps, core_ids=list(range(N_CORES)), **kw
    )


def _gather(results):
    out = np.empty((B, C_OUT, H, W), dtype=np.float32)
    for c in range(N_CORES):
        out[SPC * c: SPC * (c + 1)] = (
            results[c]["z"].reshape(SPC, C_OUT, H, W).astype(np.float32)
        )
    return out


def kernel(**inputs):
    inputs = {k: np.asarray(v) for k, v in inputs.items()}
    in_maps = _prepare_inputs(**inputs)
    res = _run(in_maps)
    return _gather(res.results)


def _install_ntff_hook():
    """Recreate antenv.axon_hooks (absent in this image) and register the
    ctypes NTFF profile hook so trace=True works under axon."""
    import sys
    import types
    try:
        import antenv
        if getattr(antenv, "axon_hooks", None) is not None:
            return
        m = types.ModuleType("antenv.axon_hooks")
        m._hook = None
        m.set_axon_ntff_profile_hook = lambda h: setattr(m, "_hook", h)
        m.get_axon_ntff_profile_hook = lambda: m._hook
        sys.modules["antenv.axon_hooks"] = m
        antenv.axon_hooks = m
        if "/root/.axon_site" not in sys.path:
            sys.path.insert(0, "/root/.axon_site")
        from trn_agent_boot.trn_boot import _ntff_profile_via_ctypes
        hook = _ntff_profile_via_ctypes("/opt/axon/libaxon_pjrt.so")
        m._hook = hook
    except Exception as e:  # profiling is best-effort
        print(f"ntff hook install failed: {e}")


def kernel_profiled(**inputs):
    _install_ntff_hook()
    inputs = {k: np.asarray(v) for k, v in inputs.items()}
    in_maps = _prepare_inputs(**inputs)
    res = _run(in_maps, trace=True, trace_cores=[0])
    return _gather(res.results), res

